# revision 15
# baseline (speedup 1.0000x reference)
"""Trainium2 Bass kernel for causal attention with relative-position bias.

Problem (hardcoded): B=16 heads, S=2048, Dh=64, fp32 I/O.
  dots = Q@K^T; bias pos=Q@R_w^T+R_b gathered by sign(j-i)+1; causal mask
  (-1e10 above diag); softmax(dots/sqrt(512)); out = probs@V.

Key algebraic simplification: within a row q the gathered bias is a constant
pos0[q] for k<q, pos1[q] at k==q (k>q is masked away). Softmax is invariant to
per-row constants, so only the diagonal needs a correction of
(pos1-pos0) = Q[q]@(R_w[1]-R_w[0]) + (R_b[1]-R_b[0]). Logits are small
(|z| <= ~2.2), so exp runs without max-subtraction and masked entries are
exact zeros.

Layout: scores computed transposed, S^T[k, q] (k on partitions), so both
matmuls use the tensor engine with contraction on partitions:
  S^T = (K^T chunk).T @ Q^T        (lhsT=K^T[64,128], rhs=Q^T[64,ncols])
  out^T[d,q] (+denominator row) = V_aug.T @ P^T   (lhsT=[V|1], rhs=exp slab)
Q^T/K^T are produced by xbar DMA transposes of fp16 casts; the [65,S] fp32
result is cast fp16, xbar-transposed back to natural layout, and divided by
the denominator.

Sharding: 16 heads -> 8 NeuronCores, 2 heads/core, no communication.
"""

import os
import sys

if "/opt/trn_rl_repo" not in sys.path:
    sys.path.insert(0, "/opt/trn_rl_repo")

import numpy as np

import concourse.bacc as bacc
import concourse.mybir as mybir
import concourse.tile as tile
from concourse.bass_utils import run_bass_kernel_spmd
from concourse.masks import make_identity, make_upper_triangular

B, S, DH = 16, 2048, 64
N_CORES = 8
HPC = B // N_CORES  # heads per core
P = 128
NT = S // P  # 16 q/k tiles per head
VW = 66  # V row width in SBUF: 64 values + ones col + pad (66*2B keeps 4B align)
OW = 80  # out^T rows padded to xbar multiple of 16 (64 vals + denom + 15 pad)
INV_SCALE = float(1.0 / np.sqrt(np.float32(512.0)))

f16 = mybir.dt.float16
f32 = mybir.dt.float32


def _emit(ctx, tc, q_d, k_d, v_d, rw_d, rb_d, out_d):
    nc = tc.nc
    AF = mybir.ActivationFunctionType

    const = ctx.enter_context(tc.tile_pool(name="const", bufs=1))
    ld = ctx.enter_context(tc.tile_pool(name="ld", bufs=2))
    hp = ctx.enter_context(tc.tile_pool(name="hp", bufs=2))
    slabp = ctx.enter_context(tc.tile_pool(name="slab", bufs=3))
    outp = ctx.enter_context(tc.tile_pool(name="outp", bufs=2))
    psc = ctx.enter_context(tc.tile_pool(name="psc", bufs=3, space="PSUM"))
    pout = ctx.enter_context(tc.tile_pool(name="pout", bufs=1, space="PSUM"))

    # constants ----------------------------------------------------------
    m01 = const.tile([P, P], f16)  # 1.0 strictly above diagonal (valid k<q)
    make_upper_triangular(nc, m01[:], val=1.0, diag=False)
    id01 = const.tile([P, P], mybir.dt.int8)
    make_identity(nc, id01[:])

    # broadcast R_w rows 0/1 and R_b[0:2] to all partitions (0-step DMA reads)
    rbc = const.tile([P, 2 * DH + 2], f32)
    nc.sync.dma_start(out=rbc[:, 0:DH], in_=rw_d[0:1, :].partition_broadcast(P))
    nc.sync.dma_start(out=rbc[:, DH : 2 * DH], in_=rw_d[1:2, :].partition_broadcast(P))
    nc.sync.dma_start(
        out=rbc[:, 2 * DH : 2 * DH + 2], in_=rb_d[None, 0:2].partition_broadcast(P)
    )
    rd16 = const.tile([P, DH], f16)  # R_w[1]-R_w[0], fp16, bcast on partitions
    nc.vector.tensor_sub(rd16[:], rbc[:, DH : 2 * DH], rbc[:, 0:DH])
    rbbias = const.tile([P, 1], f32)  # (R_b[1]-R_b[0]) / scale
    nc.vector.tensor_sub(rbbias[:], rbc[:, 2 * DH + 1 : 2 * DH + 2], rbc[:, 2 * DH : 2 * DH + 1])
    nc.vector.tensor_scalar_mul(rbbias[:], rbbias[:], INV_SCALE)

    # PE warm-up: junk matmuls at kernel start and staggered behind the
    # input-load chain, so the HAM clock gate reaches 8/8 before the main
    # loop and never sees a ~3.4us idle window on the way there.
    junk = const.tile([P, 512], f16)
    nc.gpsimd.memset(junk[:], 0.0)

    def warm_pe(count, rhs=None, parts=P):
        warm = psc.tile([P, 1024], f32, tag="sc")
        r = junk[:] if rhs is None else rhs
        for _ in range(count):
            nc.tensor.matmul(
                warm[:, 0:512], lhsT=junk[0:parts, 0:P], rhs=r, start=True,
                stop=True, skip_group_check=True,
            )

    warm_pe(12)

    for h in range(HPC):
        # load + cast to fp16 -------------------------------------------
        q32 = ld.tile([P, NT * DH], f32, tag="ld32")
        nc.sync.dma_start(
            out=q32[:].rearrange("p (n d) -> p n d", d=DH),
            in_=q_d[h].rearrange("(n p) d -> p n d", p=P),
        )
        qf = hp.tile([P, NT * DH], f16, tag="qf")
        nc.vector.tensor_copy(qf[:], q32[:])

        k32 = ld.tile([P, NT * DH], f32, tag="ld32")
        nc.sync.dma_start(
            out=k32[:].rearrange("p (n d) -> p n d", d=DH),
            in_=k_d[h].rearrange("(n p) d -> p n d", p=P),
        )
        kf = hp.tile([P, NT * DH], f16, tag="kf")
        nc.vector.tensor_copy(kf[:], k32[:])

        v32 = ld.tile([P, NT * DH], f32, tag="ld32")
        nc.sync.dma_start(
            out=v32[:].rearrange("p (n d) -> p n d", d=DH),
            in_=v_d[h].rearrange("(n p) d -> p n d", p=P),
        )
        vaug = hp.tile([P, NT * VW], f16, tag="vaug")
        v3 = vaug[:].rearrange("p (n e) -> p n e", e=VW)
        nc.gpsimd.tensor_copy(v3[:, :, 0:DH], v32[:].rearrange("p (n d) -> p n d", d=DH))
        nc.gpsimd.memset(v3[:, :, DH : DH + 1], 1.0)

        # transpose Q, K to [64, S] via xbar DMA ------------------------
        # One [128,1024]->[128,8,128] xbar transpose (logical row 128m+p
        # lands at dest[p, m, :]), then two copies unfold partitions 0:64 /
        # 64:128 (even/odd 128-col tiles) into contiguous [64, S].
        def transpose_to(src, tag):
            dst = hp.tile([DH, S], f16, tag=tag)
            fold = ld.tile([P, 8 * P], f16, tag="fold")
            nc.sync.dma_start_transpose(
                out=fold[:].rearrange("p (m r) -> p m r", r=P), in_=src[:]
            )
            d4 = dst[:].rearrange("d (m j r) -> d m j r", j=2, r=P)
            f3 = fold[:].rearrange("p (m r) -> p m r", r=P)
            nc.sync.dma_start(out=d4[:, :, 0, :], in_=f3[0:DH])
            nc.sync.dma_start(out=d4[:, :, 1, :], in_=f3[DH:P])
            return dst

        if h == 0:
            warm_pe(3, rhs=qf[:, 0:512])
            warm_pe(3, rhs=kf[:, 0:512])

        qt = transpose_to(qf, "qt")
        kt = transpose_to(kf, "kt")
        if h == 0:
            warm_pe(3, rhs=qt[:, 0:512], parts=DH)
            warm_pe(3, rhs=kt[:, 0:512], parts=DH)

        # diagonal terms: pre[q] = Q[q] . (K[q] + rdelta) ---------------
        t2 = ld.tile([P, NT * DH], f16, tag="t2")
        t2_3 = t2[:].rearrange("p (n d) -> p n d", d=DH)
        nc.vector.tensor_add(
            t2_3, kf[:].rearrange("p (n d) -> p n d", d=DH),
            rd16[:, None, :].to_broadcast([P, NT, DH]),
        )
        nc.vector.tensor_mul(t2[:], qf[:], t2[:])
        pre = hp.tile([P, NT], f32, tag="pre")
        nc.vector.tensor_reduce(
            out=pre[:], in_=t2_3, axis=mybir.AxisListType.X, op=mybir.AluOpType.add
        )
        pdiag = hp.tile([P, NT], f16, tag="pdiag")
        nc.scalar.activation(pdiag[:], pre[:], AF.Exp, bias=rbbias[:, 0:1], scale=INV_SCALE)

        # main loop: two q-phases of 1024 cols; within a phase, fills of up
        # to 1024 score columns flow QK (PE) -> exp (ACT) -> PV (PE), with
        # the QK of fill f+2 emitted before PV of fill f so the tensor
        # engine always has independent work and its HAM clock stays warm.
        outTs = outp.tile([OW, S], f16, tag="outTs")
        nc.gpsimd.memset(outTs[DH : OW, :], 0.0)
        PH = 1024  # phase width in q columns
        for ph in range(S // PH):
            lo, hi = ph * PH, (ph + 1) * PH
            # fills: one per contributing k-chunk
            fills = []
            for ki in range(NT):
                q0 = P * ki
                base = max(q0, lo)
                if base < hi:
                    fills.append((ki, q0, base, hi - base))
            outT = pout.tile([DH + 1, PH], f32, tag="outT")

            def emit_qk(f):
                ki, q0, base, n = fills[f]
                sc = psc.tile([P, 1024], f32, tag="sc")
                for so in range(0, n, 512):
                    nn = min(512, n - so)
                    nc.tensor.matmul(
                        sc[:, so : so + nn],
                        lhsT=kt[:, q0 : q0 + P],
                        rhs=qt[:, base + so : base + so + nn],
                        start=True,
                        stop=True,
                    )
                return sc

            scs = {0: emit_qk(0)}
            if len(fills) > 1:
                scs[1] = emit_qk(1)
            last_ki = fills[-1][0]
            for f, (ki, q0, base, n) in enumerate(fills):
                sc = scs.pop(f)
                slab = slabp.tile([P, 1024], f16, tag="slab")
                nc.scalar.activation(slab[:, 0:n], sc[:, 0:n], AF.Exp, scale=INV_SCALE)
                if base == q0:
                    # diagonal 128x128 block: zero k>=q, then write exp diag
                    nc.vector.tensor_mul(slab[:, 0:P], slab[:, 0:P], m01[:])
                    nc.vector.copy_predicated(
                        slab[:, 0:P], id01[:], pdiag[:, ki : ki + 1].to_broadcast([P, P])
                    )
                if f + 2 < len(fills):
                    scs[f + 2] = emit_qk(f + 2)
                for qb in range(base // 512, (base + n - 1) // 512 + 1):
                    g0 = max(base, qb * 512)
                    g1 = min(base + n, (qb + 1) * 512)
                    nc.tensor.matmul(
                        outT[:, g0 - lo : g1 - lo],
                        lhsT=v3[:, ki, 0 : DH + 1],
                        rhs=slab[:, g0 - base : g1 - base],
                        start=(ki == 0),
                        stop=(ki == min(last_ki, 4 * qb + 3)),
                        skip_group_check=True,
                    )
            nc.vector.tensor_copy(outTs[0 : DH + 1, lo:hi], outT[:, :])

        # epilogue: transpose back, divide, store -----------------------
        onat = outp.tile([P, NT * OW], f16, tag="onat")
        onat3 = onat[:].rearrange("p (n e) -> p n e", e=OW)
        nc.sync.dma_start_transpose(out=onat3, in_=outTs[:])
        recip = outp.tile([P, NT], f32, tag="recip")
        nc.vector.reciprocal(recip[:, :, None], onat3[:, :, DH : DH + 1])
        ofin = outp.tile([P, NT * DH], f32, tag="ofin")
        nc.vector.tensor_mul(
            ofin[:].rearrange("p (n d) -> p n d", d=DH),
            onat3[:, :, 0:DH],
            recip[:, :, None].to_broadcast([P, NT, DH]),
        )
        nc.sync.dma_start(
            out=out_d[h].rearrange("(n p) d -> p n d", p=P),
            in_=ofin[:].rearrange("p (n d) -> p n d", d=DH),
        )


def build_nc(debug=False):
    from contextlib import ExitStack

    nc = bacc.Bacc("TRN2", target_bir_lowering=False, debug=debug, num_devices=N_CORES)
    q_d = nc.dram_tensor("query", [HPC, S, DH], f32, kind="ExternalInput").ap()
    k_d = nc.dram_tensor("key", [HPC, S, DH], f32, kind="ExternalInput").ap()
    v_d = nc.dram_tensor("value", [HPC, S, DH], f32, kind="ExternalInput").ap()
    rw_d = nc.dram_tensor("R_w", [3, DH], f32, kind="ExternalInput").ap()
    rb_d = nc.dram_tensor("R_b", [3], f32, kind="ExternalInput").ap()
    out_d = nc.dram_tensor("out", [HPC, S, DH], f32, kind="ExternalOutput").ap()
    with tile.TileContext(nc) as tc, __import__("contextlib").ExitStack() as ctx:
        _emit(ctx, tc, q_d, k_d, v_d, rw_d, rb_d, out_d)
    nc.finalize()
    return nc


_NC_CACHE = {}


def _get_nc():
    if "nc" not in _NC_CACHE:
        _NC_CACHE["nc"] = build_nc()
    return _NC_CACHE["nc"]


def kernel(query, key, value, R_w, R_b, trace=False):
    query = np.ascontiguousarray(np.asarray(query, dtype=np.float32))
    key = np.ascontiguousarray(np.asarray(key, dtype=np.float32))
    value = np.ascontiguousarray(np.asarray(value, dtype=np.float32))
    R_w = np.ascontiguousarray(np.asarray(R_w, dtype=np.float32))
    R_b = np.ascontiguousarray(np.asarray(R_b, dtype=np.float32))

    nc = _get_nc()
    in_maps = [
        {
            "query": query[c * HPC : (c + 1) * HPC],
            "key": key[c * HPC : (c + 1) * HPC],
            "value": value[c * HPC : (c + 1) * HPC],
            "R_w": R_w,
            "R_b": R_b,
        }
        for c in range(N_CORES)
    ]
    res = run_bass_kernel_spmd(nc, in_maps, core_ids=list(range(N_CORES)), trace=trace)
    out = np.concatenate([res.results[c]["out"] for c in range(N_CORES)], axis=0)
    if trace:
        kernel.last_results = res
    return out.astype(np.float32, copy=False)


# revision 17
# speedup vs baseline: 1.2233x; 1.2233x over previous
"""Trainium2 Bass kernel for causal attention with relative-position bias.

Problem (hardcoded): B=16 heads, S=2048, Dh=64, fp32 I/O.
  dots = Q@K^T; bias pos=Q@R_w^T+R_b gathered by sign(j-i)+1; causal mask
  (-1e10 above diag); softmax(dots/sqrt(512)); out = probs@V.

Key algebraic simplification: within a row q the gathered bias is a constant
pos0[q] for k<q, pos1[q] at k==q (k>q is masked away). Softmax is invariant to
per-row constants, so only the diagonal needs a correction of
(pos1-pos0) = Q[q]@(R_w[1]-R_w[0]) + (R_b[1]-R_b[0]). Logits are small
(|z| <= ~2.2), so exp runs without max-subtraction and masked entries are
exact zeros.

Layout: scores computed transposed, S^T[k, q] (k on partitions), so both
matmuls use the tensor engine with contraction on partitions:
  S^T = (K^T chunk).T @ Q^T        (lhsT=K^T[64,128], rhs=Q^T[64,ncols])
  out^T[d,q] (+denominator row) = V_aug.T @ P^T   (lhsT=[V|1], rhs=exp slab)
Q^T/K^T are produced by xbar DMA transposes of fp16 casts; the [65,S] fp32
result is cast fp16, xbar-transposed back to natural layout, and divided by
the denominator.

Sharding: 16 heads -> 8 NeuronCores, 2 heads/core, no communication.
"""

import os
import sys

if "/opt/trn_rl_repo" not in sys.path:
    sys.path.insert(0, "/opt/trn_rl_repo")

import numpy as np

import concourse.bacc as bacc
import concourse.mybir as mybir
import concourse.tile as tile
from concourse.bass_utils import run_bass_kernel_spmd
from concourse.masks import make_identity, make_upper_triangular

B, S, DH = 16, 2048, 64
N_CORES = 8
HPC = B // N_CORES  # heads per core
P = 128
NT = S // P  # 16 q/k tiles per head
VW = 66  # V row width in SBUF: 64 values + ones col + pad (66*2B keeps 4B align)
OW = 80  # out^T rows padded to xbar multiple of 16 (64 vals + denom + 15 pad)
INV_SCALE = float(1.0 / np.sqrt(np.float32(512.0)))

f16 = mybir.dt.float16
f32 = mybir.dt.float32


def _emit(ctx, tc, q_d, k_d, v_d, rw_d, rb_d, out_d):
    nc = tc.nc
    AF = mybir.ActivationFunctionType

    const = ctx.enter_context(tc.tile_pool(name="const", bufs=1))
    ld = ctx.enter_context(tc.tile_pool(name="ld", bufs=2))
    hp = ctx.enter_context(tc.tile_pool(name="hp", bufs=2))
    slabp = ctx.enter_context(tc.tile_pool(name="slab", bufs=3))
    outp = ctx.enter_context(tc.tile_pool(name="outp", bufs=2))
    psc = ctx.enter_context(tc.tile_pool(name="psc", bufs=3, space="PSUM"))
    pout = ctx.enter_context(tc.tile_pool(name="pout", bufs=1, space="PSUM"))

    # constants ----------------------------------------------------------
    m01 = const.tile([P, P], f16)  # 1.0 strictly above diagonal (valid k<q)
    make_upper_triangular(nc, m01[:], val=1.0, diag=False)
    id01 = const.tile([P, P], mybir.dt.int8)
    make_identity(nc, id01[:])

    # broadcast R_w rows 0/1 and R_b[0:2] to all partitions (0-step DMA reads)
    rbc = const.tile([P, 2 * DH + 2], f32)
    nc.gpsimd.dma_start(out=rbc[:, 0:DH], in_=rw_d[0:1, :].partition_broadcast(P))
    nc.gpsimd.dma_start(out=rbc[:, DH : 2 * DH], in_=rw_d[1:2, :].partition_broadcast(P))
    nc.gpsimd.dma_start(
        out=rbc[:, 2 * DH : 2 * DH + 2], in_=rb_d[None, 0:2].partition_broadcast(P)
    )
    rd16 = const.tile([P, DH], f16)  # R_w[1]-R_w[0], fp16, bcast on partitions
    nc.vector.tensor_sub(rd16[:], rbc[:, DH : 2 * DH], rbc[:, 0:DH])
    rbbias = const.tile([P, 1], f32)  # (R_b[1]-R_b[0]) / scale
    nc.vector.tensor_sub(rbbias[:], rbc[:, 2 * DH + 1 : 2 * DH + 2], rbc[:, 2 * DH : 2 * DH + 1])
    nc.vector.tensor_scalar_mul(rbbias[:], rbbias[:], INV_SCALE)

    # PE warm-up: junk matmuls at kernel start and staggered behind the
    # input-load chain, so the HAM clock gate reaches 8/8 before the main
    # loop and never sees a ~3.4us idle window on the way there.
    junk = const.tile([P, 512], f16)
    nc.gpsimd.memset(junk[:], 0.0)

    def warm_pe(count, rhs=None, parts=P):
        warm = psc.tile([P, 1024], f32, tag="sc")
        r = junk[:] if rhs is None else rhs
        for _ in range(count):
            nc.tensor.matmul(
                warm[:, 0:512], lhsT=junk[0:parts, 0:P], rhs=r, start=True,
                stop=True, skip_group_check=True,
            )

    warm_pe(12)

    for h in range(HPC):
        # load + cast to fp16 -------------------------------------------
        q32 = ld.tile([P, NT * DH], f32, tag="ld32")
        nc.sync.dma_start(
            out=q32[:].rearrange("p (n d) -> p n d", d=DH),
            in_=q_d[h].rearrange("(n p) d -> p n d", p=P),
        )
        qf = hp.tile([P, NT * DH], f16, tag="qf")
        nc.vector.tensor_copy(qf[:], q32[:])

        k32 = ld.tile([P, NT * DH], f32, tag="ld32")
        nc.sync.dma_start(
            out=k32[:].rearrange("p (n d) -> p n d", d=DH),
            in_=k_d[h].rearrange("(n p) d -> p n d", p=P),
        )
        kf = hp.tile([P, NT * DH], f16, tag="kf")
        nc.vector.tensor_copy(kf[:], k32[:])

        v32 = ld.tile([P, NT * DH], f32, tag="ld32")
        nc.sync.dma_start(
            out=v32[:].rearrange("p (n d) -> p n d", d=DH),
            in_=v_d[h].rearrange("(n p) d -> p n d", p=P),
        )
        vaug = hp.tile([P, NT * VW], f16, tag="vaug")
        v3 = vaug[:].rearrange("p (n e) -> p n e", e=VW)
        nc.gpsimd.tensor_copy(v3[:, :, 0:DH], v32[:].rearrange("p (n d) -> p n d", d=DH))
        nc.gpsimd.memset(v3[:, :, DH : DH + 1], 1.0)

        # transpose Q, K to [64, S] via xbar DMA ------------------------
        # One [128,1024]->[128,8,128] xbar transpose (logical row 128m+p
        # lands at dest[p, m, :]), then two copies unfold partitions 0:64 /
        # 64:128 (even/odd 128-col tiles) into contiguous [64, S].
        # Head 0 uses the scalar HWDGE ring (ACT idle at startup) so the
        # transposes don't FIFO behind the input loads on the sync ring.
        eng = nc.scalar if h == 0 else nc.sync

        def start_transpose(src, tag):
            dst = hp.tile([DH, S], f16, tag=tag)
            fold = ld.tile([P, 8 * P], f16, tag="fold" + tag)
            eng.dma_start_transpose(
                out=fold[:].rearrange("p (m r) -> p m r", r=P), in_=src[:]
            )
            return dst, fold

        def finish_transpose(dst, fold):
            d4 = dst[:].rearrange("d (m j r) -> d m j r", j=2, r=P)
            f3 = fold[:].rearrange("p (m r) -> p m r", r=P)
            eng.dma_start(out=d4[:, :, 0, :], in_=f3[0:DH])
            eng.dma_start(out=d4[:, :, 1, :], in_=f3[DH:P])
            return dst

        if h == 0:
            warm_pe(3, rhs=qf[:, 0:512])
            warm_pe(3, rhs=kf[:, 0:512])

        qt, qfold = start_transpose(qf, "qt")
        kt, kfold = start_transpose(kf, "kt")
        qt = finish_transpose(qt, qfold)
        kt = finish_transpose(kt, kfold)

        # diagonal terms: pre[q] = Q[q] . (K[q] + rdelta) ---------------
        t2 = ld.tile([P, NT * DH], f16, tag="t2")
        t2_3 = t2[:].rearrange("p (n d) -> p n d", d=DH)
        nc.vector.tensor_add(
            t2_3, kf[:].rearrange("p (n d) -> p n d", d=DH),
            rd16[:, None, :].to_broadcast([P, NT, DH]),
        )
        nc.vector.tensor_mul(t2[:], qf[:], t2[:])
        pre = hp.tile([P, NT], f32, tag="pre")
        nc.vector.tensor_reduce(
            out=pre[:], in_=t2_3, axis=mybir.AxisListType.X, op=mybir.AluOpType.add
        )
        pdiag = hp.tile([P, NT], f16, tag="pdiag")
        nc.scalar.activation(pdiag[:], pre[:], AF.Exp, bias=rbbias[:, 0:1], scale=INV_SCALE)

        # main loop: two q-phases of 1024 cols; within a phase, fills of up
        # to 1024 score columns flow QK (PE) -> exp (ACT) -> PV (PE), with
        # the QK of fill f+2 emitted before PV of fill f so the tensor
        # engine always has independent work and its HAM clock stays warm.
        outTs = outp.tile([OW, S], f16, tag="outTs")
        nc.gpsimd.memset(outTs[DH : OW, :], 0.0)
        PH = 1024  # phase width in q columns
        for ph in range(S // PH):
            lo, hi = ph * PH, (ph + 1) * PH
            # fills: one per contributing k-chunk
            fills = []
            for ki in range(NT):
                q0 = P * ki
                base = max(q0, lo)
                if base < hi:
                    fills.append((ki, q0, base, hi - base))
            outT = pout.tile([DH + 1, PH], f32, tag="outT")

            def emit_qk(f):
                ki, q0, base, n = fills[f]
                sc = psc.tile([P, 1024], f32, tag="sc")
                for so in range(0, n, 512):
                    nn = min(512, n - so)
                    nc.tensor.matmul(
                        sc[:, so : so + nn],
                        lhsT=kt[:, q0 : q0 + P],
                        rhs=qt[:, base + so : base + so + nn],
                        start=True,
                        stop=True,
                    )
                return sc

            scs = {0: emit_qk(0)}
            if len(fills) > 1:
                scs[1] = emit_qk(1)
            last_ki = fills[-1][0]
            for f, (ki, q0, base, n) in enumerate(fills):
                sc = scs.pop(f)
                slab = slabp.tile([P, 1024], f16, tag="slab")
                nc.scalar.activation(slab[:, 0:n], sc[:, 0:n], AF.Exp, scale=INV_SCALE)
                if base == q0:
                    # diagonal 128x128 block: zero k>=q, then write exp diag
                    nc.vector.tensor_mul(slab[:, 0:P], slab[:, 0:P], m01[:])
                    nc.vector.copy_predicated(
                        slab[:, 0:P], id01[:], pdiag[:, ki : ki + 1].to_broadcast([P, P])
                    )
                if f + 2 < len(fills):
                    scs[f + 2] = emit_qk(f + 2)
                for qb in range(base // 512, (base + n - 1) // 512 + 1):
                    g0 = max(base, qb * 512)
                    g1 = min(base + n, (qb + 1) * 512)
                    nc.tensor.matmul(
                        outT[:, g0 - lo : g1 - lo],
                        lhsT=v3[:, ki, 0 : DH + 1],
                        rhs=slab[:, g0 - base : g1 - base],
                        start=(ki == 0),
                        stop=(ki == min(last_ki, 4 * qb + 3)),
                        skip_group_check=True,
                    )
            nc.vector.tensor_copy(outTs[0 : DH + 1, lo:hi], outT[:, :])

        # epilogue: transpose back, divide, store -----------------------
        onat = outp.tile([P, NT * OW], f16, tag="onat")
        onat3 = onat[:].rearrange("p (n e) -> p n e", e=OW)
        nc.sync.dma_start_transpose(out=onat3, in_=outTs[:])
        recip = outp.tile([P, NT], f32, tag="recip")
        nc.vector.reciprocal(recip[:, :, None], onat3[:, :, DH : DH + 1])
        ofin = outp.tile([P, NT * DH], f32, tag="ofin")
        nc.vector.tensor_mul(
            ofin[:].rearrange("p (n d) -> p n d", d=DH),
            onat3[:, :, 0:DH],
            recip[:, :, None].to_broadcast([P, NT, DH]),
        )
        nc.sync.dma_start(
            out=out_d[h].rearrange("(n p) d -> p n d", p=P),
            in_=ofin[:].rearrange("p (n d) -> p n d", d=DH),
        )


def build_nc(debug=False):
    from contextlib import ExitStack

    nc = bacc.Bacc("TRN2", target_bir_lowering=False, debug=debug, num_devices=N_CORES)
    q_d = nc.dram_tensor("query", [HPC, S, DH], f32, kind="ExternalInput").ap()
    k_d = nc.dram_tensor("key", [HPC, S, DH], f32, kind="ExternalInput").ap()
    v_d = nc.dram_tensor("value", [HPC, S, DH], f32, kind="ExternalInput").ap()
    rw_d = nc.dram_tensor("R_w", [3, DH], f32, kind="ExternalInput").ap()
    rb_d = nc.dram_tensor("R_b", [3], f32, kind="ExternalInput").ap()
    out_d = nc.dram_tensor("out", [HPC, S, DH], f32, kind="ExternalOutput").ap()
    with tile.TileContext(nc) as tc, __import__("contextlib").ExitStack() as ctx:
        _emit(ctx, tc, q_d, k_d, v_d, rw_d, rb_d, out_d)
    nc.finalize()
    return nc


_NC_CACHE = {}


def _get_nc():
    if "nc" not in _NC_CACHE:
        _NC_CACHE["nc"] = build_nc()
    return _NC_CACHE["nc"]


def kernel(query, key, value, R_w, R_b, trace=False):
    query = np.ascontiguousarray(np.asarray(query, dtype=np.float32))
    key = np.ascontiguousarray(np.asarray(key, dtype=np.float32))
    value = np.ascontiguousarray(np.asarray(value, dtype=np.float32))
    R_w = np.ascontiguousarray(np.asarray(R_w, dtype=np.float32))
    R_b = np.ascontiguousarray(np.asarray(R_b, dtype=np.float32))

    nc = _get_nc()
    in_maps = [
        {
            "query": query[c * HPC : (c + 1) * HPC],
            "key": key[c * HPC : (c + 1) * HPC],
            "value": value[c * HPC : (c + 1) * HPC],
            "R_w": R_w,
            "R_b": R_b,
        }
        for c in range(N_CORES)
    ]
    res = run_bass_kernel_spmd(nc, in_maps, core_ids=list(range(N_CORES)), trace=trace)
    out = np.concatenate([res.results[c]["out"] for c in range(N_CORES)], axis=0)
    if trace:
        kernel.last_results = res
    return out.astype(np.float32, copy=False)
